# revision 42
# baseline (speedup 1.0000x reference)
"""Trainium2 Bass kernel for ExtensibleAttention (sparse_attention).

Strategy: data-parallel over the 65536 tokens (N*L flattened) across 8
NeuronCores; the small 256-dim projection weights are replicated.

v2 (all-bf16): activations and weights are cast to bf16 on the host, which
halves both HBM traffic and SBUF footprint while staying well inside the
relative-error budget (measured 7.3e-3 end to end vs the 2e-2 gate; bf16
matmuls run at the same 1 cycle/row as fp32r but fp32r pays 4x below
256-wide moving data, so bf16 also makes small tail units cheap).  The PE
is the bottleneck; its work and the pipeline overheads are cut by:

  * the grid-sample y-half move (smat) is a shifted-partition Pool copy
    instead of a PE row-select matmul (engine partition ranges must start
    at 0/32/64/96 -- all shifted accesses here use 32-aligned bases);
  * e and e*w are stacked in one [64,T] tile so a single matmul computes
    both softmax partial sums (s2 in rows 0-7, s1 in rows 32-39 of the
    result, keeping every partition base 32-aligned);
  * the out-projection is emitted channel-major (stationary = Wout), and
    the host transposes the [256, T] bf16 result back to token-major;
  * all four input tensors ride in ONE packed [128,8,T] DMA per tile and
    the weights in three packs ordered by first use -- per-ring DMA issue
    costs ~625ns each, so fewer, larger DMAs shorten the critical path
    (weights go out on the ACT DGE ring, inputs/outputs on SP);
  * elementwise glue is spread across ACT / DVE / Pool so each stays
    below the PE's pace; q*k runs on the Pool (GPSIMD), which cannot
    touch PSUM, so q/k drain through DVE copies first.

The first tile runs in 2 pieces to shorten pipeline fill; the tail keeps
full 512-token units because the ~4us cross-engine chain latency of a
small unit exceeds the PE work available to hide it.
"""

import numpy as np
from contextlib import ExitStack

import ml_dtypes

import concourse.bacc as bacc
import concourse.tile as tile
from concourse import mybir

F32 = mybir.dt.float32
F32R = mybir.dt.float32r
BF16 = mybir.dt.bfloat16
AF = mybir.ActivationFunctionType

N, L, C, H, KP, D = 4, 16384, 256, 8, 4, 32
NCORES = 8
TOKS = N * L // NCORES  # 8192 tokens per core
TLOAD = 512             # tokens per DMA load tile
PIECE = 128             # token piece size for the first/last tiles
SIGMA = float(1.0 / np.sqrt(D))
NPBF = ml_dtypes.bfloat16


def _build(toks=TOKS, tload=TLOAD, with_bias=False, pos_fused=False,
           bufs_a=4, bufs_v=2, bufs_b=2, start_pieces=4, tail_units=2,
           last_split=0):
    nc = bacc.Bacc(trn_type="TRN2")
    dram = {}

    def din(name, shape, dt=BF16):
        dram[name] = nc.dram_tensor(name, list(shape), dt,
                                    kind="ExternalInput")
        return dram[name]

    # all activations packed in one tensor: rows (xq0,xq1,xp0,xp1,xk0,xk1,
    # xv0,xv1) so each load tile is a single DMA
    xall = din("xall", (128, 8, toks))
    ref = din("ref", (2, toks), F32R)
    # bf16 weights/constants in three packs ordered by first use so the
    # projection matmuls start as early as possible:
    # 1 = wq|wp (1024); 2 = wk|wv|wo1 (2048);
    # 3 = wo2 (256) | wo (512) | amat (64) | cmat2 (40) | bmat (256)
    din("wpack1", (128, 1024))
    din("wpack2", (128, 2048))
    din("wpack3", (128, 1128))
    # f32 pack: bo1 (4) | bwof (1)
    din("fpack", (128, 5), F32)
    din("pmat", (2, 64), F32R)
    if with_bias:
        din("bpack", (1, 1536))
    # channel-major output: out[p, mc, t] = channel (mc*128+p) of token t
    out = nc.dram_tensor("out", [128, 2, toks], BF16, kind="ExternalOutput")

    nload = toks // tload

    with tile.TileContext(nc) as tc, ExitStack() as ctx:
        singles = ctx.enter_context(tc.tile_pool(name="singles", bufs=1))
        inp = ctx.enter_context(tc.tile_pool(name="inp", bufs=4))
        work = ctx.enter_context(tc.tile_pool(name="work", bufs=2))
        psA = ctx.enter_context(tc.tile_pool(name="psA", bufs=bufs_a, space="PSUM"))
        psV = ctx.enter_context(tc.tile_pool(name="psV", bufs=bufs_v, space="PSUM"))
        psB = ctx.enter_context(tc.tile_pool(name="psB", bufs=bufs_b, space="PSUM"))

        mm = nc.tensor.matmul

        def load_tile(lt, pieces=1):
            t0 = lt * tload
            xall_t = inp.tile([128, 8, tload], BF16, tag="xall")
            ref_t = inp.tile([2, tload], F32R, tag="ref")
            step = tload // pieces
            for pi in range(pieces):
                s = slice(pi * step, (pi + 1) * step)
                g = slice(t0 + pi * step, t0 + (pi + 1) * step)
                nc.sync.dma_start(out=xall_t[:, :, s], in_=xall[:, :, g])
                nc.sync.dma_start(out=ref_t[:, s], in_=ref[:, g])
            xq_t = xall_t[:, 0:2, :]
            xp_t = xall_t[:, 2:4, :]
            xk_t = xall_t[:, 4:6, :]
            xv_t = xall_t[:, 6:8, :]
            return xq_t, xp_t, xk_t, xv_t, ref_t

        def stage1(ld, lo, tz, tail=False):
            """Projections, q*k product, offset MLP (drained through abs)."""
            xq_t, xp_t, xk_t, xv_t, ref_t = ld
            s = slice(lo, lo + tz)
            # at the pipeline tail the Pool's slow ops sit on the drain
            # critical path; route them to the (then idle) DVE instead
            mul_eng = nc.vector if tail else nc.gpsimd

            qp_sb = work.tile([128, 2, tz], BF16, tag="qp", bufs=2)
            kp_sb = work.tile([128, 2, tz], BF16, tag="kp", bufs=2)
            m_sb = work.tile([128, 2, tz], BF16, tag="m", bufs=3)
            for mc in range(2):
                m128 = slice(mc * 128, (mc + 1) * 128)
                if not pos_fused:
                    # pos drains to SBUF once; q/k pick it up via DVE adds
                    pos_ps = psA.tile([128, tz], F32, tag="proj")
                    mm(pos_ps, wp_s[:, 0, m128], xp_t[:, 0, s], start=True,
                       stop=False)
                    mm(pos_ps, wp_s[:, 1, m128], xp_t[:, 1, s], start=False,
                       stop=True)
                    pos_sb = work.tile([128, tz], F32, tag="pos", bufs=2)
                    nc.scalar.copy(pos_sb, pos_ps)
                q_ps = psA.tile([128, tz], F32, tag="proj")
                last_q = not with_bias and not pos_fused
                mm(q_ps, wq_s[:, 0, m128], xq_t[:, 0, s], start=True, stop=False)
                mm(q_ps, wq_s[:, 1, m128], xq_t[:, 1, s], start=False, stop=last_q)
                if pos_fused:
                    mm(q_ps, wp_s[:, 0, m128], xp_t[:, 0, s], start=False,
                       stop=False)
                    mm(q_ps, wp_s[:, 1, m128], xp_t[:, 1, s], start=False,
                       stop=not with_bias)
                if with_bias:
                    mm(q_ps, bqp_s[:, m128], ones_s[:, :tz], start=False, stop=True)
                k_ps = psA.tile([128, tz], F32, tag="proj")
                mm(k_ps, wk_s[:, 0, m128], xk_t[:, 0, s], start=True, stop=False)
                mm(k_ps, wk_s[:, 1, m128], xk_t[:, 1, s], start=False, stop=last_q)
                if pos_fused:
                    mm(k_ps, wp_s[:, 0, m128], xp_t[:, 0, s], start=False,
                       stop=False)
                    mm(k_ps, wp_s[:, 1, m128], xp_t[:, 1, s], start=False,
                       stop=not with_bias)
                if with_bias:
                    mm(k_ps, bkp_s[:, m128], ones_s[:, :tz], start=False, stop=True)
                if pos_fused:
                    nc.vector.tensor_copy(qp_sb[:, mc, :], q_ps)
                    nc.vector.tensor_copy(kp_sb[:, mc, :], k_ps)
                else:
                    nc.vector.tensor_add(qp_sb[:, mc, :], q_ps, pos_sb)
                    nc.vector.tensor_add(kp_sb[:, mc, :], k_ps, pos_sb)
                mul_eng.tensor_mul(m_sb[:, mc, :], qp_sb[:, mc, :],
                                   kp_sb[:, mc, :])

            # v = value@Wv
            v_sb = work.tile([128, 2, tz], BF16, tag="v", bufs=3)
            for mc in range(2):
                m128 = slice(mc * 128, (mc + 1) * 128)
                v_ps = psV.tile([128, tz], F32, tag="vv")
                mm(v_ps, wv_s[:, 0, m128], xv_t[:, 0, s], start=True, stop=False)
                mm(v_ps, wv_s[:, 1, m128], xv_t[:, 1, s], start=False,
                   stop=not with_bias)
                if with_bias:
                    mm(v_ps, bvr_s[:, m128], ones_s[:, :tz], start=False, stop=True)
                nc.scalar.copy(v_sb[:, mc, :], v_ps)

            # hidden = relu(query@Wo1 + bo1), 4 chunks of 128
            hid_sb = work.tile([128, 4, tz], BF16, tag="hid", bufs=2)
            for j in range(4):
                h_ps = psB.tile([128, tz], F32, tag="small")
                j128 = slice(j * 128, (j + 1) * 128)
                mm(h_ps, wo1_s[:, 0, j128], xq_t[:, 0, s], start=True, stop=False)
                mm(h_ps, wo1_s[:, 1, j128], xq_t[:, 1, s], start=False, stop=True)
                nc.scalar.activation(hid_sb[:, j, :], h_ps, AF.Relu,
                                     bias=bo1_s[:, j:j + 1], scale=1.0)

            # off = hidden@Wo2p + ref (x rows 0-31, y rows 32-63), drained
            # immediately through the Abs so the PSUM bank frees in-stage
            off_ps = psB.tile([64, tz], F32, tag="small")
            for j in range(4):
                mm(off_ps, wo2_s[:, j, :], hid_sb[:, j, :],
                   start=(j == 0), stop=False)
            mm(off_ps, pmat_s, ref_t[:, s], start=False, stop=True)
            t1_sb = work.tile([64, tz], BF16, tag="t1")
            nc.scalar.activation(t1_sb, off_ps, AF.Abs, bias=bwof_s, scale=1.0)
            return m_sb, v_sb, t1_sb, tz

        def stage2a(state, tail=False):
            """Head-sum of q*k, grid-sample weight w, softmax partial sums."""
            m_sb, v_sb, t1_sb, tz = state
            mul_eng = nc.vector if tail else nc.gpsimd

            qk_ps = psB.tile([32, tz], F32, tag="small")
            mm(qk_ps, amat_s[:, 0:32], m_sb[:, 0, :], start=True, stop=False)
            mm(qk_ps, amat_s[:, 32:64], m_sb[:, 1, :], start=False, stop=True)

            # w = relu(1-|sp_x-.5|)*relu(1-|sp_y-.5|); y half moved to
            # partitions 0-31 with a shifted-partition Pool copy
            t2_sb = work.tile([64, tz], BF16, tag="t2")
            nc.scalar.activation(t2_sb, t1_sb, AF.Relu, bias=1.0, scale=-1.0)
            t2y_sb = work.tile([32, tz], BF16, tag="t2y")
            (nc.vector if tail else nc.gpsimd).tensor_copy(t2y_sb,
                                                           t2_sb[32:64, :])
            w_sb = work.tile([32, tz], BF16, tag="w")
            mul_eng.tensor_mul(w_sb, t2_sb[0:32, :], t2y_sb)

            # softmax over K: e = exp(qk*w/sqrt(D)); e rows 0-31, e*w rows
            # 32-63 of one tile so a single matmul computes both sums;
            # s12 drains to SBUF in-stage to free its PSUM bank early
            lg_sb = work.tile([32, tz], F32, tag="lg")
            nc.vector.tensor_mul(lg_sb, qk_ps, w_sb)
            eew_sb = work.tile([64, tz], BF16, tag="eew")
            nc.scalar.activation(eew_sb[0:32, :], lg_sb, AF.Exp,
                                 bias=0.0, scale=SIGMA)
            nc.vector.tensor_mul(eew_sb[32:64, :], eew_sb[0:32, :], w_sb)
            s12_ps = psB.tile([40, tz], F32, tag="small")
            mm(s12_ps, cmat2_s, eew_sb, start=True, stop=True)
            s12_sb = work.tile([40, tz], F32, tag="s12")
            nc.scalar.copy(s12_sb, s12_ps)
            return s12_sb, v_sb, tz

        def stage2b(state, g0):
            """Softmax normalization, ov = v*wv, out-projection, store."""
            s12_sb, v_sb, tz = state
            r1_sb = work.tile([8, tz], F32, tag="r1")
            nc.vector.reciprocal(r1_sb, s12_sb[32:40, :])
            wv_sb = work.tile([8, tz], BF16, tag="wvv")
            nc.vector.tensor_mul(wv_sb, s12_sb[0:8, :], r1_sb)

            # ov = v * wv (head -> channel broadcast via bmat matmul)
            ov_sb = work.tile([128, 2, tz], BF16, tag="ov")
            for mc in range(2):
                wvx_ps = psB.tile([128, tz], F32, tag="small")
                mm(wvx_ps, bmat_s[:, mc * 128:(mc + 1) * 128], wv_sb,
                   start=True, stop=True)
                nc.vector.tensor_mul(ov_sb[:, mc, :], v_sb[:, mc, :], wvx_ps)

            # out channel-major: out[o, t] = sum_c Wout[c, o] ov[c, t];
            # each chunk DMAs as soon as its drain lands
            o_sb = work.tile([128, 2, tz], BF16, tag="osb")
            for mc in range(2):
                o_ps = psB.tile([128, tz], F32, tag="small")
                m128 = slice(mc * 128, (mc + 1) * 128)
                mm(o_ps, wo_s[:, 0, m128], ov_sb[:, 0, :], start=True, stop=False)
                mm(o_ps, wo_s[:, 1, m128], ov_sb[:, 1, :], start=False,
                   stop=not with_bias)
                if with_bias:
                    mm(o_ps, bor_s[:, m128], ones_s[:, :tz], start=False, stop=True)
                if mc == 0:
                    nc.scalar.copy(o_sb[:, mc, :], o_ps)
                    nc.sync.dma_start(out=out[:, mc, g0:g0 + tz],
                                      in_=o_sb[:, mc, :])
                else:
                    nc.vector.tensor_copy(o_sb[:, mc, :], o_ps)
                    nc.sync.dma_start(out=out[:, mc, g0:g0 + tz],
                                      in_=o_sb[:, mc, :])

        # unit list: the first load tile runs in smaller pieces to shorten
        # pipeline fill; everything else runs full 512-token units (small
        # end pieces measured worse: their ~4us cross-engine chain latency
        # exceeds the PE work available to hide it)
        units = []
        step0 = tload // start_pieces
        for pi in range(start_pieces):
            units.append((0, pi * step0, step0))
        for lt in range(1, nload - 1):
            units.append((lt, 0, tload))
        if last_split:
            units.append((nload - 1, 0, tload - last_split))
            units.append((nload - 1, tload - last_split, last_split))
        else:
            units.append((nload - 1, 0, tload))

        # 3-deep software pipeline as in v1: emit stage1(i), stage2b(i-2),
        # stage2a(i-1) so the PE stays dense while ACT/DVE chains drain
        p1 = p2 = None
        # startup order: wq|wp pack, then the xq/xp half of piece 0, then
        # the xk/xv half, then the remaining weight packs and pieces
        wpack1_s = singles.tile([128, 1024], BF16, name="wpack1_s")
        nc.scalar.dma_start(out=wpack1_s, in_=dram["wpack1"][:])
        xall0_t = inp.tile([128, 8, tload], BF16, tag="xall")
        ref0_t = inp.tile([2, tload], F32R, tag="ref")
        nc.sync.dma_start(out=xall0_t[:, 0:4, 0:step0],
                          in_=xall[:, 0:4, 0:step0])
        wpack2_s = singles.tile([128, 2048], BF16, name="wpack2_s")
        nc.scalar.dma_start(out=wpack2_s, in_=dram["wpack2"][:])
        nc.sync.dma_start(out=xall0_t[:, 4:8, 0:step0],
                          in_=xall[:, 4:8, 0:step0])
        nc.sync.dma_start(out=ref0_t[:, 0:step0], in_=ref[:, 0:step0])
        wpack3_s = singles.tile([128, 1128], BF16, name="wpack3_s")
        nc.scalar.dma_start(out=wpack3_s, in_=dram["wpack3"][:])
        fpack_s = singles.tile([128, 5], F32, name="fpack_s")
        nc.scalar.dma_start(out=fpack_s, in_=dram["fpack"][:])
        pmat_s = singles.tile([2, 64], F32R, name="pmat_s")
        nc.scalar.dma_start(out=pmat_s, in_=dram["pmat"][:])
        for pi in range(1, start_pieces):
            s = slice(pi * step0, (pi + 1) * step0)
            nc.sync.dma_start(out=xall0_t[:, :, s], in_=xall[:, :, s])
            nc.sync.dma_start(out=ref0_t[:, s], in_=ref[:, s])
        ld = (xall0_t[:, 0:2, :], xall0_t[:, 2:4, :], xall0_t[:, 4:6, :],
              xall0_t[:, 6:8, :], ref0_t)

        wq_s = wpack1_s[:, 0:512].rearrange("p (k c) -> p k c", k=2)
        wp_s = wpack1_s[:, 512:1024].rearrange("p (k c) -> p k c", k=2)
        wk_s = wpack2_s[:, 0:512].rearrange("p (k c) -> p k c", k=2)
        wv_s = wpack2_s[:, 512:1024].rearrange("p (k c) -> p k c", k=2)
        wo1_s = wpack2_s[:, 1024:2048].rearrange("p (k c) -> p k c", k=2)
        wo2_s = wpack3_s[:, 0:256].rearrange("p (k c) -> p k c", k=4)
        wo_s = wpack3_s[:, 256:768].rearrange("p (k c) -> p k c", k=2)
        amat_s = wpack3_s[:, 768:832]
        cmat2_s = wpack3_s[0:64, 832:872]
        bmat_s = wpack3_s[0:8, 872:1128]
        bo1_s = fpack_s[:, 0:4]
        bwof_s = fpack_s[0:64, 4:5]
        if with_bias:
            bpack_s = singles.tile([1, 1536], BF16, name="bpack_s")
            nc.sync.dma_start(out=bpack_s, in_=dram["bpack"][:])
            bqp_s = bpack_s[:, 0:256]
            bkp_s = bpack_s[:, 256:512]
            bvr_s = bpack_s[:, 512:768]
            bor_s = bpack_s[:, 768:1024]
            ones_s = bpack_s[:, 1024:1536]
        ld_next = None
        for ui, (lt, lo, tz) in enumerate(units):
            tail = ui >= len(units) - tail_units
            if ui + 1 < len(units) and units[ui + 1][0] != lt:
                ld_next = load_tile(units[ui + 1][0])
            state = stage1(ld, lo, tz, tail=tail)
            if p2 is not None:
                stage2b(*p2)
                p2 = None
            if p1 is not None:
                st2, g0p = p1
                p2 = (stage2a(st2, tail=tail), g0p)
            p1 = (state, lt * tload + lo)
            if ui + 1 < len(units) and units[ui + 1][0] != lt:
                ld = ld_next
        # drain: 2a of the last unit first (its chain is longest), then the
        # independent 2b of the second-to-last overlaps it
        st2, g0p = p1
        last2a = stage2a(st2, tail=True)
        if p2 is not None:
            stage2b(*p2)
        stage2b(last2a, g0p)

    nc.compile()
    return nc


def _consts():
    amat = np.zeros((128, 64), np.float32)
    for mc in range(2):
        for d in range(128):
            h = mc * 4 + d // 32
            for k in range(KP):
                amat[d, mc * 32 + h * KP + k] = 1.0
    # cmat2: rows 0-31 = e (h,k), rows 32-63 = e*w (h,k); output cols
    # 0-7 = s2 per head, cols 32-39 = s1 per head (32-aligned partition
    # bases -- engine partition ranges must start at 0/32/64/96)
    cmat2 = np.zeros((64, 40), np.float32)
    for j in range(32):
        cmat2[j, 32 + j // KP] = 1.0
        cmat2[32 + j, j // KP] = 1.0
    bmat = np.zeros((8, 256), np.float32)
    for mc in range(2):
        for c in range(128):
            bmat[mc * 4 + c // 32, mc * 128 + c] = 1.0
    pmat = np.zeros((2, 64), np.float32)
    for r in range(64):
        pmat[r // 32, r] = 1.0
    return amat, cmat2, bmat, pmat


def _wsplit(w, dt=NPBF):
    # [256, O] -> [128, 2, O]  (row kc*128+p  ->  [p, kc, :])
    o = w.shape[1]
    return np.ascontiguousarray(
        w.reshape(2, 128, o).transpose(1, 0, 2)).astype(dt)


def _xsplit(x, dt=NPBF):
    # [T, 256] token-major -> [128, 2, T] channel-major chunks
    t = x.shape[0]
    return np.ascontiguousarray(
        x.T.reshape(2, 128, t).transpose(1, 0, 2)).astype(dt)


def _host_maps(inputs, toks, ncores):
    f32 = lambda v: np.asarray(v, dtype=np.float32)
    query = f32(inputs["query"]).reshape(-1, C)
    key = f32(inputs["key"]).reshape(-1, C)
    value = f32(inputs["value"]).reshape(-1, C)
    pos = f32(inputs["pos_embed"]).reshape(-1, C)
    refp = f32(inputs["reference_points"]).reshape(-1, 2)

    # permute Wo2 columns (h,k,c) -> (c,h,k)
    perm = [h * (KP * 2) + k * 2 + c for c in range(2) for h in range(H)
            for k in range(KP)]
    wo2p = f32(inputs["Wo2"])[:, perm]
    bo2p = f32(inputs["bo2"])[perm]

    amat, cmat2, bmat, pmat = _consts()
    bqp = f32(inputs["bq"]) + f32(inputs["bpos"])
    bkp = f32(inputs["bk"]) + f32(inputs["bpos"])
    bv = f32(inputs["bv"])
    bout = f32(inputs["bout"])
    with_bias = any(np.any(b != 0) for b in (bqp, bkp, bv, bout))

    wo2r = np.ascontiguousarray(
        wo2p.reshape(4, 128, 64).transpose(1, 0, 2)).astype(NPBF)

    def flat2(w3):
        # [128, k, c] -> [128, k*c]
        return w3.reshape(128, -1)

    wpack1 = np.zeros((128, 1024), NPBF)
    wpack1[:, 0:512] = flat2(_wsplit(f32(inputs["Wq"])))
    wpack1[:, 512:1024] = flat2(_wsplit(f32(inputs["Wpos"])))
    wpack2 = np.zeros((128, 2048), NPBF)
    wpack2[:, 0:512] = flat2(_wsplit(f32(inputs["Wk"])))
    wpack2[:, 512:1024] = flat2(_wsplit(f32(inputs["Wv"])))
    wpack2[:, 1024:2048] = flat2(_wsplit(f32(inputs["Wo1"])))
    wpack3 = np.zeros((128, 1128), NPBF)
    wpack3[:, 0:256] = flat2(wo2r)
    wpack3[:, 256:768] = flat2(_wsplit(f32(inputs["Wout"])))
    wpack3[:, 768:832] = amat.astype(NPBF)
    wpack3[0:64, 832:872] = cmat2.astype(NPBF)
    wpack3[0:8, 872:1128] = bmat.astype(NPBF)

    fpack = np.zeros((128, 5), np.float32)
    fpack[:, 0:4] = f32(inputs["bo1"]).reshape(4, 128).T
    fpack[0:64, 4] = (bo2p - 0.5).reshape(64)

    shared = {"wpack1": wpack1, "wpack2": wpack2, "wpack3": wpack3,
              "fpack": fpack, "pmat": pmat}
    if with_bias:
        bpack = np.zeros((1, 1536), NPBF)
        bpack[0, 0:256] = bqp.astype(NPBF)
        bpack[0, 256:512] = bkp.astype(NPBF)
        bpack[0, 512:768] = bv.astype(NPBF)
        bpack[0, 768:1024] = bout.astype(NPBF)
        bpack[0, 1024:1536] = 1.0
        shared["bpack"] = bpack

    in_maps = []
    for cid in range(ncores):
        sl = slice(cid * toks, (cid + 1) * toks)
        m = dict(shared)
        xa = np.empty((128, 8, sl.stop - sl.start), NPBF)
        xa[:, 0:2, :] = _xsplit(query[sl])
        xa[:, 2:4, :] = _xsplit(pos[sl])
        xa[:, 4:6, :] = _xsplit(key[sl])
        xa[:, 6:8, :] = _xsplit(value[sl])
        m["xall"] = xa
        m["ref"] = np.ascontiguousarray(refp[sl].T)
        in_maps.append(m)
    return in_maps, with_bias


_NC_CACHE = {}

# best configuration found via TimelineSim sweep
BUILD_CFG = dict(pos_fused=True, bufs_a=2, bufs_v=2, bufs_b=4, start_pieces=2,
                 tail_units=2)


def kernel(**inputs):
    from concourse.bass_utils import run_bass_kernel_spmd

    in_maps, with_bias = _host_maps(inputs, TOKS, NCORES)
    ck = ("full", with_bias)
    if ck not in _NC_CACHE:
        _NC_CACHE[ck] = _build(toks=TOKS, tload=TLOAD, with_bias=with_bias,
                               **BUILD_CFG)
    nc = _NC_CACHE[ck]
    res = run_bass_kernel_spmd(nc, in_maps, core_ids=list(range(NCORES)))
    # out is channel-major [128, 2, toks] bf16 per core -> [toks, 256] f32
    outs = [np.asarray(r["out"]).astype(np.float32).transpose(2, 1, 0)
            .reshape(TOKS, C) for r in res.results]
    full = np.concatenate(outs, axis=0).reshape(N, L, C)
    return np.ascontiguousarray(full)
